# revision 40
# baseline (speedup 1.0000x reference)
"""Trainium2 Bass kernel for nn_CELossWithSVLS_VE (SVLS cross-entropy loss).

Math (derived + numerically validated vs reference):
  For the 26 non-center offsets n, with per-voxel
    u_n = exp(-0.5*(maxdiff_n^2 + r_n^2)),
    maxdiff_n(v) = max_c(img_c(v+n) - img_c(v))   (replicate-padded),
  the SVLS label weights reduce EXACTLY to w_center = 1/2, w_n = u_n/(2S),
  S = sum_n u_n.  Then
    loss(v) = lse(v) - 0.5*x_{l(v)}(v) - (1/(2S)) * sum_n u_n * x_{l(v+n)}(v)
  and the output is mean_v loss(v).

Sharding: 8 cores, core k takes d-slab [8k, 8k+8) of both batches.
On-core layout: partition p = b*64 + h (128), free = (c?, d, w) with d,w
halos in SBUF; h+-1 stencil shifts via partition-shifted SBUF copies (DMA).

Offset-pair trick: for pair +n/-n, maxdiff(+n, v) = max_c(delta_c) and
maxdiff(-n, v) = -min_c(img_c(v) - img_c(v-n)); squaring kills the sign, so
both frames are computed aligned at v (no halo compute, no shifted u arrays).
"""
import os
import sys
from contextlib import ExitStack

import numpy as np

if "/opt/trn_rl_repo" not in sys.path:
    sys.path.insert(0, "/opt/trn_rl_repo")

B, C, D, H, W = 2, 4, 64, 64, 64
NCORES = 8
DL = D // NCORES          # 8 local d-planes
DE, WE = DL + 2, W + 2    # 10, 66 (d/w halos)
P = 128                   # partitions = (b, h)
NVOX = B * D * H * W      # 524288

# 13 positive offsets; r2 = i*i+j*j+k*k sets the exp bias.
# j==0 pairs first: they need no h-shifted arrays, so the DVE can start on
# them while the partition-shift DMA copies are still in flight.
PAIRS = [
    (1, 0, 0), (0, 0, 1), (1, 0, 1), (1, 0, -1),
    (0, 1, 0), (1, 1, 0), (1, -1, 0), (0, 1, 1), (0, 1, -1),
    (1, 1, 1), (1, 1, -1), (1, -1, 1), (1, -1, -1),
]

_CACHED = {}


def _build_nc():
    import concourse.bacc as bacc
    import concourse.mybir as mybir
    import concourse.tile as tile

    AF = mybir.ActivationFunctionType
    ALU = mybir.AluOpType
    dt = mybir.dt

    nc = bacc.Bacc("TRN2", target_bir_lowering=False, debug=False,
                   num_devices=NCORES)
    img_d = nc.dram_tensor("img", [P, C * DE * WE], dt.bfloat16,
                           kind="ExternalInput")
    lab_d = nc.dram_tensor("lab", [P, DE * WE], dt.bfloat16,
                           kind="ExternalInput")
    logit_d = nc.dram_tensor("logits", [P, C * DL * W], dt.float32,
                             kind="ExternalInput")
    # mats: [I, -I, Sh(+1), Sh(-1)] stacked along free dim; Sh(j) is the
    # block-diag (per-batch) h-shift-with-edge-clamp matrix
    eye_d = nc.dram_tensor("eye", [P, 4 * P], dt.bfloat16,
                           kind="ExternalInput")
    out_d = nc.dram_tensor("partials", [P, 1], dt.float32,
                           kind="ExternalOutput")

    import concourse.bass as bass_mod

    with tile.TileContext(nc) as tc, ExitStack() as ctx:
        persist = ctx.enter_context(tc.tile_pool(name="persist", bufs=1))
        cpool = ctx.enter_context(tc.tile_pool(name="cpool", bufs=1))
        trans = ctx.enter_context(tc.tile_pool(name="trans", bufs=3))
        upool = ctx.enter_context(tc.tile_pool(name="upool", bufs=3))
        psum = ctx.enter_context(
            tc.tile_pool(name="psum", bufs=1, space=bass_mod.MemorySpace.PSUM))

        f32, bf16 = dt.float32, dt.bfloat16
        TT = nc.vector.tensor_tensor

        # ---- loads (images/labels arrive pre-cast to bf16 from host) ----
        imgb = persist.tile([P, C, DE, WE], bf16, tag="imgb")
        for c in range(C):
            nc.sync.dma_start(imgb[:, c],
                              img_d[:, c * DE * WE:(c + 1) * DE * WE])
        labf = persist.tile([P, DE, WE], bf16, tag="labf")
        nc.sync.dma_start(labf[:], lab_d[:, :])
        x = persist.tile([P, C, DL, W], f32, tag="x")
        for c in range(C):
            nc.sync.dma_start(x[:, c], logit_d[:, c * DL * W:(c + 1) * DL * W])

        masks = persist.tile([P, 3, DE, WE], bf16, tag="masks")
        for ci, cval in enumerate((1.0, 2.0, 3.0)):
            nc.vector.tensor_scalar(masks[:, ci], labf[:], cval, None,
                                    ALU.is_equal)

        # ---- h-shifted copies (partition shift via SBUF->SBUF DMA).
        # 3 DMAs per array: two 63-row block shifts + one step-64
        # partition-strided DMA covering both batches' clamped edge rows.
        def hshift_copies(dst_p, dst_m, src):
            nc.sync.dma_start(dst_p[0:63], src[1:64])
            nc.sync.dma_start(dst_p[64:127], src[65:128])
            nc.sync.dma_start(dst_p[63:64], src[63:64])
            nc.sync.dma_start(dst_p[127:128], src[127:128])
            nc.sync.dma_start(dst_m[1:64], src[0:63])
            nc.sync.dma_start(dst_m[65:128], src[64:127])
            nc.sync.dma_start(dst_m[0:1], src[0:1])
            nc.sync.dma_start(dst_m[64:65], src[64:65])

        imgb_hp = persist.tile([P, C, DE, WE], bf16, tag="imgb_hp")
        imgb_hm = persist.tile([P, C, DE, WE], bf16, tag="imgb_hm")
        hshift_copies(imgb_hp, imgb_hm, imgb)
        masks_hp = persist.tile([P, 3, DE, WE], bf16, tag="masks_hp")
        masks_hm = persist.tile([P, 3, DE, WE], bf16, tag="masks_hm")
        hshift_copies(masks_hp, masks_hm, masks)

        img_h = {1: imgb_hp, 0: imgb, -1: imgb_hm}
        msk_h = {1: masks_hp, 0: masks, -1: masks_hm}

        def cv(tile_, i, k):
            """center view shifted by (i, ., k) of a [..., DE, WE] tile."""
            return tile_[:, :, 1 + i:1 + i + DL, 1 + k:1 + k + W]

        # exp bias const tiles (-r2/2)
        bias_t = {}
        for r2 in (1.0, 2.0, 3.0):
            bt = persist.tile([P, 1], f32, tag=f"bias{int(r2)}")
            nc.gpsimd.memset(bt[:], -0.5 * r2)
            bias_t[r2] = bt

        # ---- accumulators live in PSUM; the TensorEngine does all the
        # accumulate-adds as identity matmuls (acc += I.T @ prods), freeing
        # the DVE of ~30us of serial adds.  f32 accumulation for free.
        mats = persist.tile([P, 4, P], bf16, tag="mats")
        nc.sync.dma_start(mats[:], eye_d[:, :])
        eye, negI = mats[:, 0], mats[:, 1]
        shm = {1: mats[:, 2], -1: mats[:, 3]}
        accP = psum.tile([P, 3, DL, W], f32, tag="accP")
        SP = psum.tile([P, DL, W], f32, tag="SP")
        psum2 = ctx.enter_context(
            tc.tile_pool(name="psum2", bufs=2, space=bass_mod.MemorySpace.PSUM))
        NFR = 2 * len(PAIRS)  # accumulation steps per PSUM region

        # ---- logits: lse, dx, y ----
        expx = cpool.tile([P, C, DL, W], f32, tag="expx")
        nc.scalar.activation(expx[:], x[:], AF.Exp)
        e2 = cpool.tile([P, 2, DL, W], f32, tag="e2")
        TT(e2[:], expx[:, 0:2], expx[:, 2:4], ALU.add)
        esum = cpool.tile([P, DL, W], f32, tag="esum")
        TT(esum[:], e2[:, 0], e2[:, 1], ALU.add)
        lse = cpool.tile([P, DL, W], f32, tag="lse")
        nc.scalar.activation(lse[:], esum[:], AF.Ln)

        dx = cpool.tile([P, 3, DL, W], bf16, tag="dx")
        TT(dx[:], x[:, 1:4], x[:, 0:1].broadcast_to((P, 3, DL, W)),
           ALU.subtract)

        ym = cpool.tile([P, 3, DL, W], bf16, tag="ym")
        TT(ym[:], cv(masks, 0, 0), dx[:], ALU.mult)
        yt = cpool.tile([P, DL, W], bf16, tag="yt")
        TT(yt[:], ym[:, 0], ym[:, 1], ALU.add)
        yt2 = cpool.tile([P, DL, W], bf16, tag="yt2")
        TT(yt2[:], yt[:], ym[:, 2], ALU.add)
        y = cpool.tile([P, DL, W], f32, tag="y")
        TT(y[:], yt2[:], x[:, 0], ALU.add)


        # ---- main loop: per offset-pair A-phase + exp + accumulate ----
        for pi, (i, j, k) in enumerate(PAIRS):
            r2 = float(i * i + j * j + k * k)
            m1p = trans.tile([P, 2, DL, W], bf16, tag="m1p")
            if j == 0:
                # single sub on an extended box; min-frame reads it shifted:
                # dn_n(v) = dp_n(v - n)
                nd, nw = (9 if i else 8), (65 if k else 64)
                d0, w0 = (0 if i == 1 else 1), (0 if k == 1 else 1)
                dpe = trans.tile([P, C, nd, nw], bf16, tag="dpe")
                TT(dpe[:], imgb[:, :, d0 + i:d0 + i + nd, w0 + k:w0 + k + nw],
                   imgb[:, :, d0:d0 + nd, w0:w0 + nw], ALU.subtract)
                for fr in range(2):
                    ds = 1 - d0 - (i if fr else 0)
                    ws = 1 - w0 - (k if fr else 0)
                    mop = ALU.max if fr == 0 else ALU.min
                    dv = dpe[:, :, ds:ds + DL, ws:ws + W]
                    m2 = trans.tile([P, 2, DL, W], bf16, tag="m2")
                    TT(m2[:], dv[:, 0:2], dv[:, 2:4], mop)
                    TT(m1p[:, fr], m2[:, 0], m2[:, 1], mop)
            else:
                # max-frame sub on the TensorEngine: d4 = Sh_j.T@img(i,k-view)
                # + (-I).T@img(center), accumulated in PSUM; the two
                # channel-half copybacks are exactly the L1 max-tree inputs.
                d4h = []
                for half in range(2):
                    d4p = psum2.tile([P, 2, DL, W], f32, tag="d4p")
                    for cc in range(2):
                        c = 2 * half + cc
                        nc.tensor.matmul(d4p[:, cc], shm[j],
                                         imgb[:, c, 1 + i:1 + i + DL,
                                              1 + k:1 + k + W],
                                         start=True, stop=False)
                        nc.tensor.matmul(d4p[:, cc], negI,
                                         imgb[:, c, 1:1 + DL, 1:1 + W],
                                         start=False, stop=True)
                    hh = trans.tile([P, 2, DL, W], bf16, tag="d4h")
                    nc.scalar.copy(hh[:], d4p[:])
                    d4h.append(hh)
                m2 = trans.tile([P, 2, DL, W], bf16, tag="m2")
                TT(m2[:], d4h[0][:], d4h[1][:], ALU.max)
                TT(m1p[:, 0], m2[:, 0], m2[:, 1], ALU.max)
                # min-frame stays on DVE
                d4 = trans.tile([P, C, DL, W], bf16, tag="d4")
                TT(d4[:], cv(imgb, 0, 0), cv(img_h[-j], -i, -k), ALU.subtract)
                m2n = trans.tile([P, 2, DL, W], bf16, tag="m2")
                TT(m2n[:], d4[:, 0:2], d4[:, 2:4], ALU.min)
                TT(m1p[:, 1], m2n[:, 0], m2n[:, 1], ALU.min)

            # batched square+exp for both frames (same r2 bias)
            sqp = trans.tile([P, 2, DL, W], bf16, tag="sqp")
            nc.scalar.activation(sqp[:], m1p[:], AF.Square)
            up = upool.tile([P, 2, DL, W], bf16, tag="up")
            nc.scalar.activation(up[:], sqp[:], AF.Exp,
                                 bias=bias_t[r2][:], scale=-0.5)
            uu = {0: up[:, 0], 1: up[:, 1]}

            for fr, sgn in ((0, 1), (1, -1)):
                t = 2 * pi + fr
                st, sp = (t == 0), (t == NFR - 1)
                si, sj, sk = sgn * i, sgn * j, sgn * k
                mview = cv(msk_h[sj], si, sk)
                ub = up[:, fr:fr + 1].broadcast_to((P, 3, DL, W))
                prods = trans.tile([P, 3, DL, W], bf16, tag="prods")
                TT(prods[:], ub, mview, ALU.mult)
                for ci in range(3):
                    nc.tensor.matmul(accP[:, ci], eye, prods[:, ci],
                                     start=st, stop=sp)
                nc.tensor.matmul(SP[:], eye, uu[fr], start=st, stop=sp)

        # ---- readout PSUM accumulators (ScalarE), then T and loss ----
        acc_sb = cpool.tile([P, 3, DL, W], bf16, tag="acc_sb")
        nc.scalar.copy(acc_sb[:], accP[:])
        Sf = cpool.tile([P, DL, W], f32, tag="Sf")
        nc.scalar.copy(Sf[:], SP[:])

        tp = cpool.tile([P, 3, DL, W], bf16, tag="tp")
        lse, dx, y = cph['lse'], cph['dx'], cph['y']
        TT(tp[:], acc_sb[:], dx[:], ALU.mult)
        t1 = cpool.tile([P, DL, W], bf16, tag="t1")
        TT(t1[:], tp[:, 0], tp[:, 1], ALU.add)
        t2 = cpool.tile([P, DL, W], bf16, tag="t2")
        TT(t2[:], t1[:], tp[:, 2], ALU.add)
        sx = cpool.tile([P, DL, W], f32, tag="sx")
        TT(sx[:], Sf[:], x[:, 0], ALU.mult)
        Tt = cpool.tile([P, DL, W], f32, tag="Tt")
        TT(Tt[:], t2[:], sx[:], ALU.add)

        rS = cpool.tile([P, DL, W], f32, tag="rS")
        nc.vector.reciprocal_approx_fast(rS[:], Sf[:])
        w1 = cpool.tile([P, DL, W], f32, tag="w1")
        TT(w1[:], Tt[:], rS[:], ALU.mult)
        a1 = cpool.tile([P, DL, W], f32, tag="a1")
        nc.vector.scalar_tensor_tensor(a1[:], w1[:], -0.5, lse[:],
                                       ALU.mult, ALU.add)
        losst = cpool.tile([P, DL, W], f32, tag="losst")
        partial = cpool.tile([P, 1], f32, tag="partial")
        nc.vector.scalar_tensor_tensor(losst[:], y[:], -0.5, a1[:],
                                       ALU.mult, ALU.add,
                                       accum_out=partial[:])
        nc.sync.dma_start(out_d[:, :], partial[:])

    nc.compile()
    return nc


def _get_nc():
    if "nc" not in _CACHED:
        _CACHED["nc"] = _build_nc()
    return _CACHED["nc"]


def make_in_maps(inputs, labels, images):
    """Host-side shard: full inputs -> per-core input dicts (layout prep:
    (b,h)->partition transpose, d/w halo padding, bf16 pre-cast)."""
    import ml_dtypes

    bf = ml_dtypes.bfloat16
    img = np.asarray(images, np.float32).astype(bf)
    lab = np.asarray(labels).astype(bf)  # values 0..3, exact in bf16
    lgt = np.ascontiguousarray(np.asarray(inputs, np.float32))

    img_p = np.pad(img, ((0, 0), (0, 0), (1, 1), (0, 0), (1, 1)), mode="edge")
    lab_p = np.pad(lab, ((0, 0), (1, 1), (0, 0), (1, 1)), mode="edge")

    in_maps = []
    for k in range(NCORES):
        d0 = k * DL
        ic = img_p[:, :, d0:d0 + DE]          # [2,4,10,64,66]
        lc = lab_p[:, d0:d0 + DE]             # [2,10,64,66]
        xc = lgt[:, :, d0:d0 + DL]            # [2,4,8,64,64]
        im = np.ascontiguousarray(ic.transpose(0, 3, 1, 2, 4)).reshape(P, -1)
        lm = np.ascontiguousarray(lc.transpose(0, 2, 1, 3)).reshape(P, -1)
        xm = np.ascontiguousarray(xc.transpose(0, 3, 1, 2, 4)).reshape(P, -1)
        in_maps.append({"img": im, "lab": lm, "logits": xm, "eye": _mats()})
    return in_maps


def _mats():
    """[I, -I, Sh(+1), Sh(-1)] as one [P, 4P] bf16 array. Sh(j)[k, m] = 1
    iff k = b(m)*64 + clamp(h(m)+j, 0, 63):  (Sh.T @ x)[m] = x[h+j clamped]."""
    import ml_dtypes

    eye = np.eye(P, dtype=np.float32)
    sh = {}
    for jj in (1, -1):
        M = np.zeros((P, P), np.float32)
        for m in range(P):
            b, h = divmod(m, 64)
            M[b * 64 + min(max(h + jj, 0), 63), m] = 1.0
        sh[jj] = M
    out = np.concatenate([eye, -eye, sh[1], sh[-1]], axis=1)
    return np.ascontiguousarray(out).astype(ml_dtypes.bfloat16)


def kernel(inputs, labels, images):
    from concourse.bass_utils import run_bass_kernel_spmd

    nc = _get_nc()
    in_maps = make_in_maps(inputs, labels, images)
    res = run_bass_kernel_spmd(nc, in_maps, core_ids=list(range(NCORES)))
    total = 0.0
    for k in range(NCORES):
        total += res.results[k]["partials"].astype(np.float64).sum()
    return np.float32(total / NVOX)
